# revision 1
# baseline (speedup 1.0000x reference)
"""Trainium2 Bass kernel for nn_Head: single-head self-attention where
q = k = v = x @ Wq + bq and softmax is over the *query* axis (dim 1).

Math (per batch b):
    Q = X @ Wq + bq                      [T, D]
    S = Q @ Q^T / sqrt(D)                [T, T]   (symmetric!)
    W = softmax(S, axis=0)               (normalize over rows i per column j)
    A = W^T_as_stored... A[i, d] = sum_j W[i, j] Q[j, d]

Because S is symmetric, column-softmax stats (max/sum over i for column j)
are row stats of row j.  With row-tiles R_J[p=j, f=i] = S[j, i]:
    E_J[j, i]   = exp(S[j, i] / 8)            (logits are small; no max needed)
    l_j         = sum_i E_J[j, i]
    A[i, d]     = sum_j E[j, i] * (Q[j, d] / l_j)
so   A^T = sum_J Qs_J^T @ E_J  with Qs = Q / l  — a clean accumulation, no
online-softmax rescaling.  A^T is then transposed tile-wise via the PE.

Sharding: data-parallel over batch, 2 batches per core, 8 cores, no
collectives.  Full inputs in, full output out.
"""

import numpy as np

import concourse.bass as bass
import concourse.mybir as mybir
import concourse.tile as tile
from concourse.bass import ds, ts
from concourse.bass_utils import run_bass_kernel_spmd
from concourse.masks import make_identity

# ---------------------------------------------------------------------------
# Workaround: this walrus build rejects more than one sync-wait command per
# instruction.  After Tile scheduling, split any instruction carrying N>1 sem
# waits into N-1 preceding single-wait EventSemaphore instructions on the same
# (in-order) engine queue, leaving one wait on the original instruction.
# ---------------------------------------------------------------------------


def split_multi_waits(nc: bass.Bass) -> int:
    import bass_rust

    n_split = 0
    for f in nc.m.functions:
        for blk in f.blocks:
            insts = blk.instructions
            if not any(
                i.sync_info is not None and len(i.sync_info.on_wait) > 1
                for i in insts
            ):
                continue
            new_list = []
            for ins in insts:
                si = ins.sync_info
                if si is not None and len(si.on_wait) > 1:
                    waits = list(si.on_wait)
                    for k, w in enumerate(waits[:-1]):
                        e = mybir.InstEventSemaphore(
                            name=f"wsplit_{ins.name}_{k}", ins=[], outs=[]
                        )
                        e.engine = ins.engine
                        e.sync_info = bass_rust.SyncInfo(on_wait=[w], on_update=[])
                        new_list.append(e)
                        n_split += 1
                    si.on_wait = waits[-1:]
                new_list.append(ins)
            blk.instructions = new_list
    return n_split

# ---------------------------------------------------------------------------

B, T, E, D = 16, 2048, 512, 64
NCORES = 8
BPC = B // NCORES  # batches per core
P = 128
NJ = T // P  # 16 row-tiles
NCH = T // 512  # 4 512-wide column chunks

f32 = mybir.dt.float32
f32r = mybir.dt.float32r
bf16 = mybir.dt.bfloat16
EXP = mybir.ActivationFunctionType.Exp


def build_module(reps: int = 1) -> bass.Bass:
    nc = bass.Bass("TRN2", target_bir_lowering=False, debug=False, num_devices=NCORES)
    x = nc.declare_dram_parameter("x", [BPC, T, E], f32, isOutput=False).ap()
    wq = nc.declare_dram_parameter("Wq", [E, D], f32, isOutput=False).ap()
    bq = nc.declare_dram_parameter("bq", [D], f32, isOutput=False).ap()
    out = nc.declare_dram_parameter("out", [BPC, T, D], f32, isOutput=True).ap()
    xbf = nc.dram_tensor("xbf", [BPC, T, E], bf16).ap()

    with tile.TileContext(nc) as tc:
        with (
            tc.tile_pool(name="consts", bufs=1) as consts,
            tc.tile_pool(name="xt_p", bufs=2) as xt_p,
            tc.tile_pool(name="qt_p", bufs=2) as qt_p,
            tc.tile_pool(name="qn_p", bufs=2) as qn_p,
            tc.tile_pool(name="qs_p", bufs=2) as qs_p,
            tc.tile_pool(name="e_p", bufs=NJ + 2) as e_p,
            tc.tile_pool(name="l_p", bufs=8) as l_p,
            tc.tile_pool(name="at_p", bufs=2) as at_p,
            tc.tile_pool(name="o_p", bufs=4) as o_p,
            tc.tile_pool(name="ps_s", bufs=2, space="PSUM") as ps_s,
            tc.tile_pool(name="ps_m", bufs=2, space="PSUM") as ps_m,
        ):
            # constants
            wq_f = consts.tile([P, E // P, D], f32)
            nc.gpsimd.dma_start(out=wq_f[:], in_=wq.rearrange("(ko p) d -> p ko d", p=P))
            wq_b = consts.tile([P, E // P, D], bf16)
            nc.vector.tensor_copy(wq_b[:], wq_f[:])
            bq_sb = consts.tile([D, 1], f32)
            nc.gpsimd.dma_start(out=bq_sb[:], in_=bq.unsqueeze(1))
            ident = consts.tile([D, D], f32)
            make_identity(nc, ident[:])
            ident_r = consts.tile([D, D], f32r)
            nc.vector.tensor_copy(ident_r[:], ident[:])

            for rep in range(reps):
              for b in range(BPC):
                # X cast to bf16 in DRAM, then xbar-transposed load:
                # xt[p, ko, t] = x[b, t, ko*128 + p]
                nc.gpsimd.dma_start(out=xbf[b], in_=x[b])
                xt = xt_p.tile([P, E // P, T], bf16, tag="xt", name=f"xt{b}")
                nc.sync.dma_start(out=xt[:], in_=xbf[b], transpose=True)

                # projection: QT[d, t] = sum_e Wq[e, d] x[t, e]  (+ bq)
                qt = qt_p.tile([D, T], f32r, tag="qt", name=f"qt{b}")
                for c in range(NCH):
                    ps = ps_s.tile([D, 512], f32, tag="small", name=f"pj{b}_{c}")
                    for ko in range(E // P):
                        nc.tensor.matmul(
                            ps[:],
                            lhsT=wq_b[:, ko, :],
                            rhs=xt[:, ko, ts(c, 512)],
                            start=(ko == 0),
                            stop=(ko == E // P - 1),
                        )
                    nc.vector.tensor_scalar_add(qt[:, ts(c, 512)], ps[:], bq_sb[:])

                # Q natural [t, d] via PE transposes of QT tiles
                qn = qn_p.tile([P, NJ, D], f32, tag="qn", name=f"qn{b}")
                for j in range(NJ):
                    pt = ps_s.tile([P, D], f32r, tag="small", name=f"ptq{b}_{j}")
                    nc.tensor.transpose(pt[:], qt[:, ts(j, P)], ident_r[:])
                    nc.vector.tensor_copy(qn[:, j, :], pt[:].bitcast(f32))

                # phase A: per row-tile J compute S, exp, l, Qs
                qs = qs_p.tile([P, NJ, D], bf16, tag="qs", name=f"qs{b}")
                e_tiles = []
                for j in range(NJ):
                    et = e_p.tile([P, T], bf16, tag="E", name=f"e{b}_{j}")
                    e_tiles.append(et)
                    l_parts = []
                    for h in range(2):
                        ps = ps_m.tile([P, 1024], f32, tag="s", name=f"s{b}_{j}_{h}")
                        for c in range(2):
                            nc.tensor.matmul(
                                ps[:, ts(c, 512)],
                                lhsT=qt[:, ts(j, P)],
                                rhs=qt[:, ds(h * 1024 + c * 512, 512)],
                                start=True,
                                stop=True,
                            )
                        lp = l_p.tile([P, 1], f32, tag="l", name=f"lp{b}_{j}_{h}")
                        nc.scalar.activation(
                            et[:, ds(h * 1024, 1024)],
                            ps[:],
                            EXP,
                            bias=0.0,
                            scale=0.125,
                            accum_out=lp[:],
                        )
                        l_parts.append(lp)
                    rj = l_p.tile([P, 1], f32, tag="l", name=f"r{b}_{j}")
                    nc.vector.tensor_add(rj[:], l_parts[0][:], l_parts[1][:])
                    nc.vector.reciprocal(rj[:], rj[:])
                    nc.vector.tensor_scalar_mul(qs[:, j, :], qn[:, j, :], rj[:])

                # phase B: A^T[d, i] = sum_J Qs_J^T @ E_J, per 512-col chunk
                at = at_p.tile([D, T], f32, tag="at", name=f"at{b}")
                for c in range(NCH):
                    ps = ps_s.tile([D, 512], f32, tag="small", name=f"pa{b}_{c}")
                    for j in range(NJ):
                        nc.tensor.matmul(
                            ps[:],
                            lhsT=qs[:, j, :],
                            rhs=e_tiles[j][:, ts(c, 512)],
                            start=(j == 0),
                            stop=(j == NJ - 1),
                        )
                    nc.vector.tensor_copy(at[:, ts(c, 512)], ps[:])

                # transpose A^T tile-wise back to [t, d] and store
                for j in range(NJ):
                    pt = ps_s.tile([P, D], f32, tag="small", name=f"pto{b}_{j}")
                    nc.tensor.transpose(pt[:], at[:, ts(j, P)], ident[:])
                    ot = o_p.tile([P, D], f32, tag="o", name=f"o{b}_{j}")
                    nc.vector.tensor_copy(ot[:], pt[:])
                    nc.sync.dma_start(out=out[b, ts(j, P)], in_=ot[:])

    split_multi_waits(nc)
    return nc


def kernel(x: np.ndarray, Wq: np.ndarray, bq: np.ndarray) -> np.ndarray:
    assert x.shape == (B, T, E) and Wq.shape == (E, D) and bq.shape == (D,)
    nc = build_module()
    in_maps = [
        {
            "x": np.ascontiguousarray(x[i * BPC : (i + 1) * BPC]),
            "Wq": np.ascontiguousarray(Wq),
            "bq": np.ascontiguousarray(bq),
        }
        for i in range(NCORES)
    ]
    res = run_bass_kernel_spmd(nc, in_maps, core_ids=list(range(NCORES)))
    return np.concatenate([res.results[i]["out"] for i in range(NCORES)], axis=0)



# revision 6
# speedup vs baseline: 1093.4623x; 1093.4623x over previous
"""Trainium2 Bass kernel (v2) for nn_Head: single-head self-attention with
q = k = v = x @ Wq + bq and softmax over the *query* axis (dim 1).

Math per batch:
    QT = Wq^T X^T + bq                          [D, T]
    S_J = Q_J Q^T  (row-tile J)                 [128, T]
    E_J = exp(S_J / 8),  l_J = rowsum(E_J)      (S symmetric -> column-softmax
                                                 stats are row stats of E_J)
    A^T = sum_J (Q_J / l_J)^T E_J               [D, T]

Structure (vs the v1 baseline):
  - X^T, Q(natural) and A(natural) produced by XBAR transpose DMAs instead of
    64/16/16 PE transposes + PSUM round-trips; the input cast+transpose is
    chunked (2 half casts, 4 quarter transposes) to shorten the pipeline fill.
  - One fused exp per S half-tile on the ACT engine; for half of them the
    softmax row-sum comes from a DVE tensor_reduce instead of the ACT
    accumulator read, splitting the reduction cost across two engines.
  - l_J depends only on row-tile J (symmetry), so qs_J = Q_J/l_J is formed
    immediately (reciprocal on DVE, small tensor ops on GPSIMD) and phase B
    is software-pipelined across batches: half 0 of batch b's A^T
    accumulation interleaves into slots 8..15 of b's own S loop, half 1 into
    slots 0..7 of the next batch's loop.
  - PSUM: 3 x [128,1024] S tiles (6 banks, also reused by the projection
    chunks) + 1 x [64,1024] A^T half accumulator (2 banks) = exactly 8 banks.
  - Q is pre-scaled by GAMMA on the host (weights/bias only), undone in the
    A^T copy; the ACT exp applies scale = 0.125/GAMMA^2.  (GAMMA is a
    leftover degree of freedom from a custom-DVE exp path that this walrus
    build cannot compile; it is numerically neutral.)
  - output via one casting SWDGE DMA per batch.

Sharding: data-parallel over batch, 2 batches per core, 8 cores, no
collectives.  Full inputs in, full output out.
"""

import numpy as np

import concourse.bass as bass
import concourse.mybir as mybir
import concourse.tile as tile
from concourse.bass import ds, ts
from concourse.bass_utils import run_bass_kernel_spmd

B, T, E, D = 16, 2048, 512, 64
NCORES = 8
BPC = B // NCORES
P = 128
NJ = T // P  # 16
# (j, half) S-subtiles (16 of 32) whose softmax row-sum runs on the DVE
# halves whose row-sum runs on the DVE (tensor_reduce) instead of the ACT
# accumulator; custom DVE exp ops fail codegen on this walrus build.
DVE_JH = frozenset((j, h) for j in range(NJ) for h in (0, 1) if (j + h) % 2)

f32 = mybir.dt.float32
bf16 = mybir.dt.bfloat16
EXP = mybir.ActivationFunctionType.Exp
COPY = mybir.ActivationFunctionType.Copy

# ---------------------------------------------------------------------------
# Workaround (carried over from v1): this walrus build rejects >1 sync-wait
# per instruction; split extras into EventSemaphore preludes.
# ---------------------------------------------------------------------------


def split_multi_waits(nc: bass.Bass) -> int:
    import bass_rust

    n_split = 0
    for f in nc.m.functions:
        for blk in f.blocks:
            insts = blk.instructions
            if not any(
                i.sync_info is not None and len(i.sync_info.on_wait) > 1
                for i in insts
            ):
                continue
            new_list = []
            for ins in insts:
                si = ins.sync_info
                if si is not None and len(si.on_wait) > 1:
                    waits = list(si.on_wait)
                    for k, w in enumerate(waits[:-1]):
                        e = mybir.InstEventSemaphore(
                            name=f"wsplit_{ins.name}_{k}", ins=[], outs=[]
                        )
                        e.engine = ins.engine
                        e.sync_info = bass_rust.SyncInfo(on_wait=[w], on_update=[])
                        new_list.append(e)
                        n_split += 1
                    si.on_wait = waits[-1:]
                new_list.append(ins)
            blk.instructions = new_list
    return n_split


# ---------------------------------------------------------------------------
# exp-path scaling constants (see docstring note on GAMMA)
# ---------------------------------------------------------------------------

# near-minimax deg-3 fit of e^(logits/64) -> out^2 = q^8 ~ exp(logits/8) to
# ~0.6% rel over logits in [-4.8, 4.8].  The final squaring happens in a
# tensor_tensor_reduce which also produces the softmax row sums.  The fit's
# constant EC0 != 1 cancels exactly in the softmax (E and l scale together
# within a row-tile).
# ---------------------------------------------------------------------------

# near-minimax deg-3 fit of e^y on |y| <= 0.62 (rel err 7.6e-4)
EC0, EC1, EC2, EC3 = 0.99933327, 1.00165659, 0.51559608, 0.16348972
GAMMA = float(np.sqrt(EC1 / 64.0))  # Q prescale so psum = EC1*logits/8
ACT_SCALE = 0.125 / (GAMMA * GAMMA)  # recovers logits/8 for the ACT exp path

# ---------------------------------------------------------------------------


def build_module(reps: int = 1) -> bass.Bass:
    nc = bass.Bass("TRN2", target_bir_lowering=False, debug=False, num_devices=NCORES)
    x = nc.declare_dram_parameter("x", [BPC, T, E], f32, isOutput=False).ap()
    wq = nc.declare_dram_parameter("Wq", [E, D], f32, isOutput=False).ap()
    bq = nc.declare_dram_parameter("bq", [D], f32, isOutput=False).ap()
    out = nc.declare_dram_parameter("out", [BPC, T, D], f32, isOutput=True).ap()
    xbf = nc.dram_tensor("xbf", [BPC, T, E], bf16).ap()

    with tile.TileContext(nc) as tc:
        with (
            tc.tile_pool(name="consts", bufs=1) as consts,
            tc.tile_pool(name="xt_p", bufs=4) as xt_p,
            tc.tile_pool(name="qt_p", bufs=2) as qt_p,
            tc.tile_pool(name="qn_p", bufs=2) as qn_p,
            tc.tile_pool(name="qs_p", bufs=NJ + 4) as qs_p,
            tc.tile_pool(name="e_p", bufs=NJ + 4) as e_p,
            tc.tile_pool(name="l_p", bufs=8) as l_p,
            tc.tile_pool(name="at_p", bufs=2) as at_p,
            tc.tile_pool(name="an_p", bufs=2) as an_p,
            tc.tile_pool(name="ps_s", bufs=3, space="PSUM") as ps_s,
            tc.tile_pool(name="ps_a", bufs=1, space="PSUM") as ps_a,
        ):
            # constants
            wq_f = consts.tile([P, E // P, D], f32)
            nc.gpsimd.dma_start(out=wq_f[:], in_=wq.rearrange("(ko p) d -> p ko d", p=P))
            wq_b = consts.tile([P, E // P, D], bf16)
            nc.vector.tensor_copy(wq_b[:], wq_f[:])
            bq_sb = consts.tile([D, 1], f32)
            nc.gpsimd.dma_start(out=bq_sb[:], in_=bq.unsqueeze(1))

            QT4 = T // 4  # 512

            def emit_cast(rep, b, t2):
                # cast one T-half of x[b] to bf16 in DRAM
                nc.gpsimd.dma_start(
                    out=xbf[b, ds(t2 * (T // 2), T // 2)],
                    in_=x[b, ds(t2 * (T // 2), T // 2)],
                )

            def emit_xt_quarter(st, q):
                rep, b = st["rep"], st["b"]
                xt = xt_p.tile([P, E // P, QT4], bf16, tag="xt", name=f"xt{rep}_{b}_{q}")
                nc.sync.dma_start(
                    out=xt[:], in_=xbf[st["b"], ds(q * QT4, QT4)], transpose=True
                )
                st["xt"].append(xt)

            def emit_proj_chunk(st, c):
                rep, b = st["rep"], st["b"]
                if st["qtb"] is None:
                    st["qtb"] = qt_p.tile([D, T], bf16, tag="qtb", name=f"qtb{rep}_{b}")
                pst = ps_s.tile([P, 1024], f32, tag="s", name=f"psq{rep}_{b}_{c}")
                psq = pst[0:D, 0:512]
                for ko in range(E // P):
                    nc.tensor.matmul(
                        psq,
                        lhsT=wq_b[:, ko, :],
                        rhs=st["xt"][c][:, ko, :],
                        start=(ko == 0),
                        stop=(ko == E // P - 1),
                    )
                nc.vector.tensor_scalar_add(st["qtb"][:, ts(c, 512)], psq, bq_sb[:])

            def emit_qn(st):
                rep, b = st["rep"], st["b"]
                st["qn"] = qn_p.tile([P, NJ, D], bf16, tag="qn", name=f"qn{rep}_{b}")
                nc.sync.dma_start(out=st["qn"][:], in_=st["qtb"][:], transpose=True)

            def emit_sj(st, j):
                """S row-tile j -> E_j (exp on ACT or DVE) -> l_j -> qs_j."""
                rep, b, qtb, qn = st["rep"], st["b"], st["qtb"], st["qn"]
                et = e_p.tile([P, T], bf16, tag="E", name=f"e{rep}_{b}_{j}")
                lj = l_p.tile([P, 2], f32, tag="l", name=f"l{rep}_{b}_{j}")
                for h in range(2):
                    ps = ps_s.tile([P, 1024], f32, tag="s", name=f"s{rep}_{b}_{j}_{h}")
                    for c in range(2):
                        nc.tensor.matmul(
                            ps[:, ts(c, 512)],
                            lhsT=qtb[:, ts(j, P)],
                            rhs=qtb[:, ds(h * 1024 + c * 512, 512)],
                            start=True,
                            stop=True,
                        )
                    if (j, h) not in DVE_JH:
                        nc.scalar.activation(
                            et[:, ds(h * 1024, 1024)],
                            ps[:],
                            EXP,
                            bias=0.0,
                            scale=ACT_SCALE,
                            accum_out=lj[:, ds(h, 1)],
                        )
                    else:
                        nc.scalar.activation(
                            et[:, ds(h * 1024, 1024)],
                            ps[:],
                            EXP,
                            bias=0.0,
                            scale=ACT_SCALE,
                        )
                        nc.vector.tensor_reduce(
                            lj[:, ds(h, 1)],
                            et[:, ds(h * 1024, 1024)],
                            mybir.AxisListType.X,
                            mybir.AluOpType.add,
                        )
                rj = l_p.tile([P, 1], f32, tag="r", name=f"r{rep}_{b}_{j}")
                nc.gpsimd.tensor_add(rj[:], lj[:, ds(0, 1)], lj[:, ds(1, 1)])
                nc.vector.reciprocal(rj[:], rj[:])
                qsj = qs_p.tile([P, D], bf16, tag="qs", name=f"qs{rep}_{b}_{j}")
                nc.gpsimd.tensor_scalar_mul(qsj[:], qn[:, j, :], rj[:])
                st["et"].append(et)
                st["qs"].append(qsj)

            def emit_bhalf_begin(st, half):
                rep, b = st["rep"], st["b"]
                st["psa"] = ps_a.tile(
                    [D, 1024], f32, tag="psa", name=f"psa{rep}_{b}_{half}"
                )

            def emit_bmms(st, half, j2s):
                for j2 in j2s:
                    for c in range(2):
                        nc.tensor.matmul(
                            st["psa"][:, ts(c, 512)],
                            lhsT=st["qs"][j2][:],
                            rhs=st["et"][j2][:, ds(half * 1024 + c * 512, 512)],
                            start=(j2 == 0),
                            stop=(j2 == NJ - 1),
                        )

            def emit_bhalf_end(st, half):
                rep, b = st["rep"], st["b"]
                if st["at"] is None:
                    st["at"] = at_p.tile([D, T], bf16, tag="at", name=f"at{rep}_{b}")
                nc.vector.tensor_scalar_mul(
                    st["at"][:, ds(half * 1024, 1024)], st["psa"][:], 1.0 / GAMMA
                )
                st["psa"] = None

            def emit_out(st):
                rep, b = st["rep"], st["b"]
                an = an_p.tile([P, NJ, D], bf16, tag="an", name=f"an{rep}_{b}")
                nc.sync.dma_start(out=an[:], in_=st["at"][:], transpose=True)
                nc.gpsimd.dma_start(
                    out=out[st["b"]].rearrange("(j p) d -> p j d", p=P), in_=an[:]
                )

            def new_state(rep, b):
                return {
                    "rep": rep, "b": b, "xt": [], "qtb": None, "qn": None,
                    "et": [], "qs": [], "at": None, "psa": None,
                }

            def emit_load_all(st):
                emit_cast(st["rep"], st["b"], 0)
                emit_xt_quarter(st, 0)
                emit_xt_quarter(st, 1)
                emit_cast(st["rep"], st["b"], 1)
                emit_xt_quarter(st, 2)
                emit_xt_quarter(st, 3)
                for c in range(4):
                    emit_proj_chunk(st, c)
                emit_qn(st)

            # ---- software-pipelined emission over the batch stream ----
            # j-loop slots 0..15 per batch: S/exp/qs; slots 8..15 also carry
            # this batch's phase-B half 0 AND the next batch's load+proj;
            # slots 0..7 of the next batch carry this batch's half 1.
            slots = [(rep, b) for rep in range(reps) for b in range(BPC)]
            prev = None
            cur = new_state(*slots[0])
            emit_load_all(cur)
            for idx in range(len(slots)):
                nxt = new_state(*slots[idx + 1]) if idx + 1 < len(slots) else None
                for j in range(NJ):
                    if nxt is not None and j == 4:
                        emit_cast(nxt["rep"], nxt["b"], 0)
                        emit_xt_quarter(nxt, 0)
                        emit_xt_quarter(nxt, 1)
                        emit_cast(nxt["rep"], nxt["b"], 1)
                        emit_xt_quarter(nxt, 2)
                        emit_xt_quarter(nxt, 3)
                    emit_sj(cur, j)
                    if j < 8:
                        if prev is not None:
                            if j == 0:
                                emit_bhalf_begin(prev, 1)
                            emit_bmms(prev, 1, [2 * j, 2 * j + 1])
                            if j == 7:
                                emit_bhalf_end(prev, 1)
                                emit_out(prev)
                                prev = None
                    else:
                        if j == 8:
                            emit_bhalf_begin(cur, 0)
                        emit_bmms(cur, 0, [2 * (j - 8), 2 * (j - 8) + 1])
                        if j == NJ - 1:
                            emit_bhalf_end(cur, 0)
                if nxt is not None:
                    for c in range(4):
                        emit_proj_chunk(nxt, c)
                    emit_qn(nxt)
                prev, cur = cur, nxt
            # drain the last batch
            emit_bhalf_begin(prev, 1)
            emit_bmms(prev, 1, list(range(NJ)))
            emit_bhalf_end(prev, 1)
            emit_out(prev)

    split_multi_waits(nc)
    return nc


def make_in_maps(x: np.ndarray, Wq: np.ndarray, bq: np.ndarray):
    """Per-core input shards; Wq/bq pre-scaled by GAMMA (see EXP4Q_ANT)."""
    wq_s = np.ascontiguousarray(Wq * GAMMA, dtype=np.float32)
    bq_s = np.ascontiguousarray(bq * GAMMA, dtype=np.float32)
    return [
        {
            "x": np.ascontiguousarray(x[i * BPC : (i + 1) * BPC]),
            "Wq": wq_s,
            "bq": bq_s,
        }
        for i in range(NCORES)
    ]


def kernel(x: np.ndarray, Wq: np.ndarray, bq: np.ndarray) -> np.ndarray:
    assert x.shape == (B, T, E) and Wq.shape == (E, D) and bq.shape == (D,)
    nc = build_module()
    in_maps = make_in_maps(x, Wq, bq)
    res = run_bass_kernel_spmd(nc, in_maps, core_ids=list(range(NCORES)))
    return np.concatenate([res.results[i]["out"] for i in range(NCORES)], axis=0)
